# revision 21
# baseline (speedup 1.0000x reference)
"""Trainium2 Bass kernel for nn_CustomSelfAttention (sparse-bias attention).

Sharding (8 cores): 4 head-groups (3 heads each) x 2 query-halves (2048 each).
Each core computes its heads' attention for its query half in S^T layout
(keys on partitions, queries on free dim).

Design (vs the f32r baseline):
- All matmul operands are bf16 (1 cycle/column on the PE vs ~2 for f32r;
  PSUM accumulation stays f32). Halves PE time and x/weight DMA.
- The sparse attention bias is applied MULTIPLICATIVELY after exp:
  softmax(s/8 + c) uses P = exp(s/8) * e^c. ACT evicts score PSUM with a
  fused exp straight to fp16 SBUF; DVE multiplies by a host-built e^count
  matrix in its 4x all-SBUF/16-bit perf mode. This removes the f32
  bias-add pass that dominated DVE (242us) in the baseline.
- h-outer loop per query block: one live attnV accumulator bank instead of
  three, freeing PSUM for 3-deep score buffering so the PE streams without
  stalls (and can hold its 2.4GHz p-state).
- Softmax normalization is off the critical path: the [65,512] accumulator
  (64 out rows + a ones-row denominator) is evicted to SBUF f32 by Pool,
  then reciprocal_approx_fast + partition_broadcast + one DVE mul produce
  the normalized fp16 out-proj stationary whenever engines are free.
- Out-projection runs as a small tail phase (PSUM is free then), partials
  summed on host across head groups; bq enters exactly as a per-key ACT
  bias (host-computed beta), bk is softmax-invariant and dropped exactly,
  bv/bo are exact host-side post-adds.

Host key-rotation trick: for odd cores the key axis (xT columns, e^count
rows) is rotated so the core's own query half is always columns 0:2048 --
softmax is permutation-invariant over keys, and one SPMD program serves
all 8 cores.
"""

import numpy as np

# problem shapes (hardcoded per contract)
B, N, E, H, D = 1, 4096, 768, 12, 64
NG, NS = 4, 2           # head-group axis x query-half axis = 8 cores
HG = H // NG            # 3 heads per group
DG = HG * D             # 192
Q = N // NS             # 2048 queries per core
KC = N // 128           # 32 key chunks
EC = E // 128           # 6 contraction chunks for projections
TB = N // 512           # 8 token blocks
QB = Q // 512           # 4 query blocks per core
SCALE = float(D) ** -0.5

_prog_cache = {}


def _legalize_waits(nc, mybir, max_waits=1):
    """Split multi-wait sync_info into preceding 1-wait NoOps (TRN2 TPB
    instructions encode a single sem-wait slot; this walrus build rejects
    more)."""
    counter = 0
    n_split = 0
    for bb in nc.main_func.blocks:
        out = []
        changed = False
        for inst in bb.instructions:
            si = getattr(inst, "sync_info", None)
            if si is not None and si.on_wait and len(si.on_wait) > max_waits:
                waits = list(si.on_wait)
                for w in waits[:-max_waits]:
                    counter += 1
                    nop = mybir.InstNoOp(
                        name=f"legalize-nop-{id(nc)}-{counter}", ins=[], outs=[]
                    )
                    nop.engine = inst.engine
                    nop.sync_info = mybir.SyncInfo(on_wait=[w], on_update=[])
                    nop.bass_nofuse = True
                    try:
                        nc.register_instruction(nop, overwrite=True)
                    except Exception:
                        pass
                    out.append(nop)
                inst.sync_info = mybir.SyncInfo(
                    on_wait=waits[-max_waits:], on_update=si.on_update
                )
                n_split += 1
                changed = True
            out.append(inst)
        if changed:
            bb.instructions = out
    return n_split


def _build_program(has_bq):
    import concourse.bass as bass
    import concourse.mybir as mybir
    import concourse.tile as tile

    F32 = mybir.dt.float32
    F16 = mybir.dt.bfloat16
    EXP = mybir.ActivationFunctionType.Exp

    nc = bass.Bass(target_bir_lowering=False, debug=True)

    xT = nc.dram_tensor("xT", [E, N], F16, kind="ExternalInput")
    # weights pre-packed on host into SBUF partition-major layout so each
    # DMA is 128 fat descriptors instead of ~768 thin ones
    wkv = nc.dram_tensor("wkv", [128, EC * HG * 128], F16, kind="ExternalInput")
    wq = nc.dram_tensor("wq", [128, EC * DG], F16, kind="ExternalInput")
    wo = nc.dram_tensor("wo", [64, HG * E], F16, kind="ExternalInput")
    et = nc.dram_tensor("et", [N, Q], F16, kind="ExternalInput")
    ident = nc.dram_tensor("ident", [64, 64], F16, kind="ExternalInput")
    if has_bq:
        betad = nc.dram_tensor("beta", [N, HG], F32, kind="ExternalInput")
    outp = nc.dram_tensor("outp", [Q, E], F32, kind="ExternalOutput")

    with tile.TileContext(nc) as tc:
        with tc.tile_pool(name="persist", bufs=1) as persist:
            # --- resident weights/constants ---
            wkv_sb = persist.tile([128, EC, HG * 128], F16)
            wq_sb = persist.tile([128, EC, DG], F16)
            wo_sb = persist.tile([64, HG, E], F16)
            id_sb = persist.tile([64, 64], F16)
            nc.sync.dma_start(
                out=wkv_sb, in_=wkv[:, :].rearrange("p (c n) -> p c n", c=EC))
            nc.sync.dma_start(
                out=wq_sb, in_=wq[:, :].rearrange("p (c n) -> p c n", c=EC))
            nc.sync.dma_start(
                out=wo_sb, in_=wo[:, :].rearrange("p (h n) -> p h n", h=HG))
            nc.sync.dma_start(out=id_sb, in_=ident[:, :])
            if has_bq:
                beta_sb = persist.tile([128, KC, HG], F32)
                nc.sync.dma_start(
                    out=beta_sb,
                    in_=betad[:, :].rearrange("(c p) h -> p c h", p=128))

            # K^T / Q^T per head; V-hat [keys, d|1] per chunk with ones col
            kT = [persist.tile([64, N], F16, tag=f"kT{h}", name=f"kT{h}")
                  for h in range(HG)]
            qT = [persist.tile([64, Q], F16, tag=f"qT{h}", name=f"qT{h}")
                  for h in range(HG)]
            vt = persist.tile([128, KC, HG, 65], F16)
            nc.vector.memset(vt[:, :, :, 64:65], 1.0)
            ones3 = persist.tile([65, 64], F16)
            nc.vector.memset(ones3, 1.0)

            # unnormalized attention output + denominator, f32, per (b, h)
            ou = [[persist.tile([65, 512], F32, tag=f"ou{b}_{h}", name=f"ou{b}_{h}")
                   for h in range(HG)] for b in range(QB)]
            # normalized out-proj stationaries, per (b, h) so interleaved
            # out-projection only depends on its own block's normalize
            otn = [[persist.tile([64, 512], F16, tag=f"otn{b}_{h}",
                                 name=f"otn{b}_{h}")
                    for h in range(HG)] for b in range(QB)]

            # E-matrix (e^count) slabs per query block, double buffered.
            # Slabs 2/3 are prefetched mid-attention (on the idle Pool DMA
            # queue) so their buffer-recycle waits never block other DMAs.
            with tc.tile_pool(name="epool", bufs=2) as epool:
                esb = [None] * QB

                def load_esb(b, eng):
                    t = epool.tile([128, KC, 512], F16, tag="esb", name=f"esb{b}")
                    # split the 4.2MB slab into 4 DMAs for queue interleaving
                    for p4 in range(4):
                        eng.dma_start(
                            out=t[:, 8 * p4:8 * (p4 + 1), :],
                            in_=et[1024 * p4:1024 * (p4 + 1),
                                   512 * b:512 * (b + 1)]
                            .rearrange("(c p) q -> p c q", p=128))
                    esb[b] = t

                # ---------- projections ----------
                with tc.tile_pool(name="pj_kv", bufs=4, space="PSUM") as pj_kv, \
                     tc.tile_pool(name="pj_tr", bufs=2, space="PSUM") as pj_tr, \
                     tc.tile_pool(name="xstream", bufs=3) as xstream, \
                     tc.tile_pool(name="vtmp_pool", bufs=3) as vtmp_pool:
                    for tb in range(TB):
                        xs = xstream.tile([128, EC, 512], F16, tag="xs",
                                          name=f"xs{tb}")
                        nc.sync.dma_start(
                            out=xs,
                            in_=xT[:, 512 * tb:512 * (tb + 1)]
                            .rearrange("(c p) q -> p c q", p=128))
                        akv = [pj_kv.tile([128, 512], F32, tag="akv", name="akv")
                               for _ in range(HG)]
                        own = tb < QB  # own-half tokens: also project Q
                        if own:
                            aq = pj_kv.tile([128, 512], F32, tag="aq", bufs=1,
                                            name="aq")
                            aq2 = pj_kv.tile([64, 512], F32, tag="aq2", bufs=1,
                                             name="aq2")
                        for ec in range(EC):
                            for h in range(HG):
                                nc.tensor.matmul(
                                    akv[h],
                                    wkv_sb[:, ec, 128 * h:128 * (h + 1)],
                                    xs[:, ec, :],
                                    start=(ec == 0), stop=(ec == EC - 1))
                            if own:
                                nc.tensor.matmul(aq, wq_sb[:, ec, 0:128],
                                                 xs[:, ec, :],
                                                 start=(ec == 0), stop=(ec == EC - 1))
                                nc.tensor.matmul(aq2, wq_sb[:, ec, 128:192],
                                                 xs[:, ec, :],
                                                 start=(ec == 0), stop=(ec == EC - 1))
                        if own:
                            sl = slice(512 * tb, 512 * (tb + 1))
                            nc.scalar.copy(qT[0][:, sl], aq[0:64, :])
                            nc.scalar.copy(qT[1][:, sl], aq[64:128, :])
                            nc.scalar.copy(qT[2][:, sl], aq2)
                        for h in range(HG):
                            nc.scalar.copy(
                                kT[h][:, 512 * tb:512 * (tb + 1)], akv[h][0:64, :])
                            vtmp = vtmp_pool.tile([64, 512], F16, tag="vtmp",
                                                  name="vtmp")
                            nc.vector.tensor_copy(vtmp, akv[h][64:128, :])
                            for c4 in range(0, 4, 2):
                                ptr = pj_tr.tile([128, 2, 64], F16, tag="ptr",
                                                 name="ptr")
                                nc.tensor.transpose(
                                    ptr[:, 0, :], vtmp[:, 128 * c4:128 * (c4 + 1)],
                                    id_sb)
                                nc.tensor.transpose(
                                    ptr[:, 1, :],
                                    vtmp[:, 128 * (c4 + 1):128 * (c4 + 2)], id_sb)
                                c = tb * 4 + c4
                                nc.vector.tensor_copy(vt[:, c:c + 2, h, 0:64], ptr)
                        # E-slab DMAs overlap projection compute, not startup
                        if tb == 1:
                            load_esb(0, nc.gpsimd)
                        elif tb == 5:
                            load_esb(1, nc.gpsimd)

                # ---------- attention (h-outer per query block) ----------
                # Normalize + out-projection of block b-1 are queued as small
                # PE "filler" groups and interleaved into block b's granule
                # stream: the PE never idles waiting for ACT (exp is the rate
                # limiter), so it keeps its high p-state, and there is no
                # serial out-projection tail except for the last block.
                with tc.tile_pool(name="ps_sc", bufs=3, space="PSUM") as ps_sc, \
                     tc.tile_pool(name="ps_oa", bufs=1, space="PSUM") as ps_oa, \
                     tc.tile_pool(name="ps_sh", bufs=1, space="PSUM") as ps_sh, \
                     tc.tile_pool(name="xpool", bufs=3) as xpool, \
                     tc.tile_pool(name="tpool", bufs=3) as tpool, \
                     tc.tile_pool(name="npool", bufs=2) as npool, \
                     tc.tile_pool(name="opool", bufs=3) as opool:

                    fillers = []
                    osb_tiles = {}
                    rec16s = {}

                    def filler_norm_a(b):
                        # batch the slow iterative reciprocal across all 3
                        # heads (cost is free-size-bound, so 1x instead of 3x)
                        def run():
                            dn = npool.tile([65, 512], F32, tag="dn", bufs=2,
                                            name="dn")
                            for h in range(HG):
                                nc.vector.tensor_copy(
                                    dn[32 * h:32 * h + 1, :], ou[b][h][64:65, :])
                            rec = npool.tile([65, 512], F32, tag="rec", bufs=2,
                                             name="rec")
                            with nc.allow_low_precision(reason="softmax denom"):
                                nc.vector.reciprocal(rec, dn)
                            rec16 = npool.tile([65, 512], F16, tag="rec16",
                                               bufs=2, name="rec16")
                            nc.vector.tensor_copy(rec16, rec)
                            rec16s[b] = rec16
                        return run

                    def filler_norm_b(b, h):
                        def run():
                            rec16 = rec16s[b]
                            rbp = ps_sh.tile([128, 512], F32, tag="sh", bufs=1,
                                             name="rbp")
                            nc.tensor.matmul(
                                rbp[0:64, :], ones3[32 * h:32 * h + 1, :],
                                rec16[32 * h:32 * h + 1, :],
                                start=True, stop=True)
                            nc.vector.tensor_mul(
                                otn[b][h], ou[b][h][0:64, :], rbp[0:64, :])
                        return run

                    def filler_po(b, t, half):
                        # out-projection for queries [512b+128t : +128),
                        # E-columns half 0: [0,512) / half 1: [512,768)
                        def run():
                            e0, e1 = (0, 512) if half == 0 else (512, 768)
                            po = ps_sh.tile([128, 512], F32, tag="sh", bufs=1,
                                            name="po")
                            tsl = slice(128 * t, 128 * (t + 1))
                            for h in range(HG):
                                nc.tensor.matmul(
                                    po[:, 0:e1 - e0], otn[b][h][:, tsl],
                                    wo_sb[:, h, e0:e1],
                                    start=(h == 0), stop=(h == HG - 1))
                            if half == 0:
                                osb = opool.tile([128, E], F32, tag="osb",
                                                 name="osb")
                                osb_tiles[(b, t)] = osb
                            else:
                                osb = osb_tiles.pop((b, t))
                            nc.vector.tensor_copy(
                                osb[:, e0:e1], po[:, 0:e1 - e0])
                            if half == 1:
                                qrow = 512 * b + 128 * t
                                nc.sync.dma_start(
                                    out=outp[qrow:qrow + 128, :], in_=osb)
                        return run

                    def queue_block_work(b):
                        fillers.append(filler_norm_a(b))
                        for h in range(HG):
                            fillers.append(filler_norm_b(b, h))
                        for t in range(4):
                            for half in range(2):
                                fillers.append(filler_po(b, t, half))

                    for b in range(QB):
                        qsl = slice(512 * b, 512 * (b + 1))
                        for h in range(HG):
                            oaug = ps_oa.tile([65, 512], F32, tag="oaug",
                                              name="oaug")
                            for cp in range(KC // 2):
                                ps = ps_sc.tile([128, 2, 512], F32, tag="ps",
                                                name="ps")
                                for j in range(2):
                                    c = 2 * cp + j
                                    nc.tensor.matmul(
                                        ps[:, j, :],
                                        kT[h][:, 128 * c:128 * (c + 1)],
                                        qT[h][:, qsl],
                                        start=True, stop=True)
                                pexp = xpool.tile([128, 2, 512], F16, tag="pexp",
                                                  name="pexp")
                                if has_bq:
                                    for j in range(2):
                                        c = 2 * cp + j
                                        nc.scalar.activation(
                                            pexp[:, j, :], ps[:, j, :], EXP,
                                            scale=SCALE,
                                            bias=beta_sb[:, c, h:h + 1])
                                else:
                                    nc.scalar.activation(pexp, ps, EXP,
                                                         scale=SCALE)
                                pt = tpool.tile([128, 2, 512], F16, tag="pt",
                                                name="pt")
                                nc.vector.tensor_mul(
                                    pt, pexp, esb[b][:, 2 * cp:2 * cp + 2, :])
                                for j in range(2):
                                    c = 2 * cp + j
                                    nc.tensor.matmul(
                                        oaug, vt[:, c, h, :], pt[:, j, :],
                                        start=(c == 0), stop=(c == KC - 1))
                                # one filler per 4 granules keeps added PE
                                # work inside the ACT-bound slack
                                if cp % 4 == 3 and fillers:
                                    fillers.pop(0)()
                            nc.vector.tensor_copy(ou[b][h], oaug)
                        if b + 2 < QB:
                            load_esb(b + 2, nc.gpsimd)
                        queue_block_work(b)
                    while fillers:
                        fillers.pop(0)()

    _legalize_waits(nc, mybir)
    return nc


def _host_prep(inputs):
    import ml_dtypes

    F16 = ml_dtypes.bfloat16

    x = np.asarray(inputs["x"], dtype=np.float32)[0]          # [N, E]
    sm = np.asarray(inputs["similarity_matrix"]).astype(np.int64)  # [N, 5, 2]
    Wq = np.asarray(inputs["Wq"], dtype=np.float32)
    bq = np.asarray(inputs["bq"], dtype=np.float32)
    Wk = np.asarray(inputs["Wk"], dtype=np.float32)
    Wv = np.asarray(inputs["Wv"], dtype=np.float32)
    Wo = np.asarray(inputs["Wo"], dtype=np.float32)

    has_bq = bool(np.any(bq != 0.0))
    xT = np.ascontiguousarray(x.T)                            # [E, N]

    # dense count matrix -> e^count (multiplicative softmax bias),
    # [keys, queries] orientation
    idx = sm.reshape(N, -1)
    vals = np.where(idx < N, 1.0, 0.0).astype(np.float32)
    safe = np.minimum(idx, N - 1)
    Bm = np.zeros((N, N), dtype=np.float32)
    np.add.at(Bm, (np.repeat(np.arange(N), idx.shape[1]), safe.reshape(-1)),
              vals.reshape(-1))
    EtT = np.exp(Bm.T, dtype=np.float32)                      # [keys, queries]

    in_maps = []
    for core in range(8):
        g, s = core // NS, core % NS
        gcols = slice(g * DG, (g + 1) * DG)
        wq_np = np.ascontiguousarray(Wq[gcols, :].T)          # [E, 192]
        wkv_np = np.zeros((E, HG * 128), dtype=np.float32)
        beta_np = np.zeros((N, HG), dtype=np.float32)
        for h in range(HG):
            hc = slice((g * HG + h) * D, (g * HG + h + 1) * D)
            wkv_np[:, 128 * h:128 * h + 64] = Wk[hc, :].T
            wkv_np[:, 128 * h + 64:128 * h + 128] = Wv[hc, :].T
            if has_bq:
                beta_np[:, h] = x @ (Wk[hc, :].T @ bq[hc])    # exact bq term
        wo_np = np.ascontiguousarray(Wo[:, gcols].T)          # [192, E]

        # rotate the key axis so this core's query half is columns 0:Q
        if s == 0:
            xr = xT
            er = np.ascontiguousarray(EtT[:, 0:Q])
            br = beta_np
        else:
            xr = np.concatenate([xT[:, Q:], xT[:, :Q]], axis=1)
            er = np.concatenate([EtT[Q:, Q:], EtT[:Q, Q:]], axis=0)
            br = np.concatenate([beta_np[Q:], beta_np[:Q]], axis=0)

        wkv_pk = wkv_np.reshape(EC, 128, HG * 128).transpose(1, 0, 2)
        wq_pk = wq_np.reshape(EC, 128, DG).transpose(1, 0, 2)
        wo_pk = wo_np.reshape(HG, 64, E).transpose(1, 0, 2)
        m = {
            "xT": np.ascontiguousarray(xr).astype(F16),
            "wkv": np.ascontiguousarray(wkv_pk).reshape(128, -1).astype(F16),
            "wq": np.ascontiguousarray(wq_pk).reshape(128, -1).astype(F16),
            "wo": np.ascontiguousarray(wo_pk).reshape(64, -1).astype(F16),
            "et": np.ascontiguousarray(er).astype(F16),
            "ident": np.eye(64, dtype=np.float32).astype(F16),
        }
        if has_bq:
            m["beta"] = np.ascontiguousarray(br)
        in_maps.append(m)
    return in_maps, has_bq


def kernel(**inputs):
    from concourse.bass_utils import run_bass_kernel_spmd

    in_maps, has_bq = _host_prep(inputs)
    key = ("prog", has_bq)
    if key not in _prog_cache:
        _prog_cache[key] = _build_program(has_bq)
    nc = _prog_cache[key]

    res = run_bass_kernel_spmd(nc, in_maps, list(range(8)))

    bv = np.asarray(inputs["bv"], dtype=np.float32)
    bo = np.asarray(inputs["bo"], dtype=np.float32)
    Wo = np.asarray(inputs["Wo"], dtype=np.float32)

    full = np.zeros((N, E), dtype=np.float32)
    for core in range(8):
        s = core % NS
        full[s * Q:(s + 1) * Q, :] += res.results[core]["outp"]
    full += (bv @ Wo.T + bo)[None, :]
    return full.reshape(B, N, E)
